# revision 6
# baseline (speedup 1.0000x reference)

"""Causal attention (no head split) on 8 trn2 NeuronCores — v3.

Reference computation (per batch b):
    q = x @ Wq^T ; k = x @ Wk^T ; v = x @ Wv^T          (nn.Linear convention)
    wei = softmax(mask(q @ k^T / sqrt(C)))               (causal)
    out = wei @ v

Algebraic restructuring (K and V are never materialized):
    S   = q k^T = x (Wq^T Wk) x^T = x M x^T     with M precomputed on host
    out = wei v = (wei x) Wv^T
Device computes, all PE operands bf16:
    G^T = M^T xq^T                   (projection of this core's queries)
    S^T[s,t] = xT(lhsT) G^T(rhs)     (contract over C)
    P^T = exp(S^T / 32) * mask ; per-chunk rowsum partials -> DRAM
    H[c,t] += xn(lhsT) P^T(rhs)      (contract over s, accumulated in SBUF)
    O[q,d]  = hr(lhsT) Wv^T(rhs)     (contract over c, natural-layout out)
Final softmax normalization (divide by summed rowsum) happens on the host.

Sharding: 2 cores per batch (B=4). Queries split into eight 256-row strips;
role A takes strips {0,2,4,6} (rows [512j,512j+256)), role B {1,3,5,7}.
Every core runs the IDENTICAL instruction stream (single SPMD NEFF); role
differences are carried entirely by input data (query columns + mask tiles).

v3 layout: key chunks processed in natural order; for chunk c the
participating query strips are a CONTIGUOUS column range [256*(c//2), 1024),
processed in strip-aligned pieces of width <= 512 so S/H matmuls run at
N=512 where possible.  H-accumulation adds alternate Vector/GpSimd.  The
G phase is split (strips 0,1 / strips 2,3) and interleaved with the first
chunks to hide DMA pacing.  Per-strip Wv projections run at the strip's
last chunk (odd c), spreading output copies/DMA through the kernel.
"""
import os
import numpy as np

import concourse.bass as bass
from concourse import bacc
import concourse.mybir as mybir
from concourse.tile import TileContext
from concourse import bass_utils

B, T, C = 4, 2048, 1024
P = 128
CS = C // P          # 8 contraction subtiles
NCH = T // 256       # 8 kv chunks of 256
QS = 4               # query strips per core
SW = 256             # strip width
SCALE = 1.0 / np.sqrt(C)  # 1/32

BF16 = mybir.dt.bfloat16
F32 = mybir.dt.float32


def chunk_pieces(c):
    """Strip-aligned pieces (q0, width<=512) covering [256*(c//2), 1024)."""
    q0 = 256 * (c // 2)
    if q0 == 0:
        return [(0, 512), (512, 512)]
    if q0 == 256:
        return [(256, 256), (512, 512)]
    if q0 == 512:
        return [(512, 512)]
    return [(768, 256)]


def build():
    nc = bacc.Bacc(trn_type="TRN2", name="causal_attn_v3")
    xT = nc.dram_tensor("xT", [C, T], BF16, kind="ExternalInput")    # x^T (batch)
    xn = nc.dram_tensor("xn", [T, C], BF16, kind="ExternalInput")    # x natural
    xqT = nc.dram_tensor("xqT", [C, QS * SW], BF16, kind="ExternalInput")
    wm = nc.dram_tensor("wm", [C, C], BF16, kind="ExternalInput")    # M = Wq^T Wk
    wvT = nc.dram_tensor("wvT", [C, C], BF16, kind="ExternalInput")  # Wv^T [c,d]
    masks = nc.dram_tensor("masks", [P, 4, SW], BF16, kind="ExternalInput")
    ones = nc.dram_tensor("ones", [P, 1], BF16, kind="ExternalInput")
    oo = nc.dram_tensor("oo", [QS * 2 * P, C], F32, kind="ExternalOutput")
    rows = nc.dram_tensor("rows", [QS, QS * SW], F32, kind="ExternalOutput")

    xT_r = xT.rearrange("(cs p) t -> p cs t", p=P)
    xn_r = xn.rearrange("(ch ss p) c -> p ch ss c", p=P, ss=2)
    xqT_r = xqT.rearrange("(cs p) t -> p cs t", p=P)
    wm_r = wm.rearrange("(cs p) d -> p cs d", p=P)
    wvT_r = wvT.rearrange("(cs p) d -> p cs d", p=P)
    oo_r = oo.rearrange("(j qh p) d -> p j qh d", qh=2, p=P)

    with TileContext(nc) as tc:
        with tc.tile_pool(name="psS", bufs=4, space="PSUM") as psS, \
             tc.tile_pool(name="psH", bufs=3, space="PSUM") as psH, \
             tc.tile_pool(name="psR", bufs=1, space="PSUM") as psR, \
             tc.tile_pool(name="keep", bufs=1) as keep, \
             tc.tile_pool(name="wpool", bufs=2) as wpool, \
             tc.tile_pool(name="stream", bufs=8) as stream, \
             tc.tile_pool(name="hrpool", bufs=2) as hrpool, \
             tc.tile_pool(name="ppool", bufs=4) as ppool, \
             tc.tile_pool(name="opool", bufs=3) as opool:

            gT = keep.tile([P, CS, QS * SW], BF16, tag="gT")
            hh = keep.tile([P, CS, QS * SW], F32, tag="hh")
            msk = keep.tile([P, 4, SW], BF16, tag="msk")
            ones_t = keep.tile([P, 1], BF16, tag="ones")

            # ---- DMA order: wm first (G consumes all of it within ~7us),
            # then query strips 0,1; fine splits spread early loads over the
            # 16 queues.  wv goes last (first needed at project(0)).
            wq = wpool.tile([P, CS, C], BF16, tag="w")
            for cs in range(CS):
                for dh in range(2):
                    nc.sync.dma_start(wq[:, cs, dh * 512:(dh + 1) * 512],
                                      wm_r[:, cs, dh * 512:(dh + 1) * 512])
            xq = keep.tile([P, CS, QS * SW], BF16, tag="xq")
            for cs in range(CS):
                nc.sync.dma_start(xq[:, cs, 0:512], xqT_r[:, cs, 0:512])
            nc.sync.dma_start(msk[:], masks[:])
            nc.sync.dma_start(ones_t[:], ones[:])
            for cs in range(CS):
                nc.sync.dma_start(xq[:, cs, 512:1024], xqT_r[:, cs, 512:1024])

            wv = wpool.tile([P, CS, C], BF16, tag="w")
            for cs in range(CS):
                nc.sync.dma_start(wv[:, cs, :], wvT_r[:, cs, :])

            def g_phase(j):
                # gT[:, ds, 256j:+256] = M^T xq^T for strip j
                for ds in range(CS):
                    pq = psH.tile([P, SW], F32, tag="po")
                    for cs in range(CS):
                        nc.tensor.matmul(
                            pq[:], wq[:, cs, ds * P:(ds + 1) * P],
                            xq[:, cs, j * SW:(j + 1) * SW],
                            start=(cs == 0), stop=(cs == CS - 1))
                    nc.scalar.copy(gT[:, ds, j * SW:(j + 1) * SW], pq[:])

            def load_chunk(c):
                xt = stream.tile([P, CS, 256], BF16, tag="xt")
                for h in range(4):
                    nc.sync.dma_start(
                        xt[:, 2 * h:2 * h + 2],
                        xT_r[:, 2 * h:2 * h + 2, c * 256:(c + 1) * 256])
                xna = stream.tile([P, 2, C], BF16, tag="xn")
                for ss in range(2):
                    nc.sync.dma_start(xna[:, ss], xn_r[:, c, ss])
                return xt, xna

            def s_piece(c, xt, q0, w, first_piece):
                # S^T then P^T = exp(S/32) [* mask on the diagonal 256 cols]
                pT = ppool.tile([P, 2, w], BF16, tag="pT")
                for ss in range(2):
                    st = psS.tile([P, w], F32, tag="st")
                    for cs in range(CS):
                        nc.tensor.matmul(
                            st[:], xt[:, cs, ss * P:(ss + 1) * P],
                            gT[:, cs, q0:q0 + w],
                            start=(cs == 0), stop=(cs == CS - 1))
                    nc.scalar.activation(
                        pT[:, ss], st[:],
                        mybir.ActivationFunctionType.Exp, scale=float(SCALE))
                if first_piece:
                    # first 256 cols of the first piece = diagonal strip c//2
                    midx = c % 2
                    nc.gpsimd.tensor_mul(
                        pT[:, :, 0:SW], pT[:, :, 0:SW],
                        msk[:, midx * 2:midx * 2 + 2])
                return pT

            def h_pair(pair, xna0, xna1, pT0, pT1, q0, w):
                # rowsum partials for the chunk pair -> DRAM (summed on host)
                rw = psR.tile([1, w], F32, tag="rw")
                for i, pT in enumerate((pT0, pT1)):
                    for ss in range(2):
                        nc.tensor.matmul(
                            rw[:], ones_t[:], pT[:, ss],
                            start=(i == 0 and ss == 0),
                            stop=(i == 1 and ss == 1))
                rsb = opool.tile([1, w], F32, tag="rsb")
                nc.scalar.copy(rsb[:], rw[:])
                nc.sync.dma_start(rows[pair:pair + 1, q0:q0 + w], rsb[:])

                # H += x P^T accumulated over both chunks of the pair in PSUM
                for cs in range(CS):
                    po = psH.tile([P, w], F32, tag="po")
                    for i, (xna, pT) in enumerate(((xna0, pT0), (xna1, pT1))):
                        for ss in range(2):
                            nc.tensor.matmul(
                                po[:], xna[:, ss, cs * P:(cs + 1) * P],
                                pT[:, ss],
                                start=(i == 0 and ss == 0),
                                stop=(i == 1 and ss == 1))
                    hsl = hh[:, cs, q0:q0 + w]
                    if pair == 0:
                        nc.vector.tensor_copy(hsl, po[:])
                    else:
                        nc.vector.tensor_add(hsl, hsl, po[:])

            def project(j):
                # O[q, d] = hr(lhsT) @ Wv^T(rhs); hr cast per cs-half so the
                # first contraction steps start before the second half lands
                hr = hrpool.tile([P, CS, SW], BF16, tag="hr")
                tsl = slice(j * SW, (j + 1) * SW)
                for h2 in range(2):
                    nc.scalar.copy(hr[:, 4 * h2:4 * h2 + 4],
                                   hh[:, 4 * h2:4 * h2 + 4, tsl])
                for qh in range(2):
                    for dp in range(2):
                        ps = psH.tile([P, 512], F32, tag="po")
                        for cs in range(CS):
                            nc.tensor.matmul(
                                ps[:], hr[:, cs, qh * P:(qh + 1) * P],
                                wv[:, cs, dp * 512:(dp + 1) * 512],
                                start=(cs == 0), stop=(cs == CS - 1))
                        ost = opool.tile([P, 512], F32, tag="ost")
                        nc.scalar.copy(ost[:], ps[:])
                        nc.sync.dma_start(
                            oo_r[:, j, qh, dp * 512:(dp + 1) * 512], ost[:])

            # ---- schedule: chunk pairs (strip boundaries align to pairs);
            # G(0,1) | pair0-A | G(2,3) | projJ0 | pair0-B | pair1 | projJ1 |
            # pair2 | projJ2 | pair3 | projJ3
            g_phase(0)
            g_phase(1)
            xt0, xna0 = load_chunk(0)
            xt1, xna1 = load_chunk(1)
            pA0 = s_piece(0, xt0, 0, 512, True)
            pA1 = s_piece(1, xt1, 0, 512, True)
            h_pair(0, xna0, xna1, pA0, pA1, 0, 512)
            g_phase(2)
            g_phase(3)
            project(0)
            pB0 = s_piece(0, xt0, 512, 512, False)
            pB1 = s_piece(1, xt1, 512, 512, False)
            h_pair(0, xna0, xna1, pB0, pB1, 512, 512)
            for pair in (1, 2, 3):
                c0, c1 = 2 * pair, 2 * pair + 1
                xta, xnaa = load_chunk(c0)
                xtb, xnab = load_chunk(c1)
                for (q0, w) in chunk_pieces(c0):
                    first = (q0 == 256 * pair)
                    pa = s_piece(c0, xta, q0, w, first)
                    pb = s_piece(c1, xtb, q0, w, first)
                    h_pair(pair, xnaa, xnab, pa, pb, q0, w)
                project(pair)

    nc.compile()
    return nc


_NC = None


def _get_nc():
    global _NC
    if _NC is None:
        _NC = build()
    return _NC


def make_in_maps(x, Wq, Wk, Wv):
    import ml_dtypes
    bf16 = ml_dtypes.bfloat16
    x = np.asarray(x, dtype=np.float32)
    wq64 = np.asarray(Wq, np.float64)
    wk64 = np.asarray(Wk, np.float64)
    wm = (wq64.T @ wk64).astype(bf16)                        # M = Wq^T Wk [c',c]
    wvT = np.ascontiguousarray(np.asarray(Wv, np.float32).T).astype(bf16)
    ones = np.ones((P, 1), bf16)

    # mask tiles [p, midx*2+ss, t]: tri = 1 if (ss*128+p) <= t
    s_idx = (np.arange(2)[:, None, None] * P + np.arange(P)[None, :, None])
    tri = (s_idx <= np.arange(SW)[None, None, :]).astype(np.float32)
    tri = np.ascontiguousarray(tri.transpose(1, 0, 2))
    zeros = np.zeros((P, 2, SW), np.float32)
    ones2 = np.ones((P, 2, SW), np.float32)
    mask_A = np.ascontiguousarray(np.concatenate([tri, zeros], axis=1)).astype(bf16)
    mask_B = np.ascontiguousarray(np.concatenate([ones2, tri], axis=1)).astype(bf16)

    xr = [x[b].astype(bf16) for b in range(B)]
    xT = [np.ascontiguousarray(xr[b].T) for b in range(B)]
    in_maps = []
    for core in range(8):
        b, role = divmod(core, 2)
        cols = np.concatenate(
            [np.arange(512 * j + SW * role, 512 * j + SW * role + SW)
             for j in range(QS)])
        xqT = np.ascontiguousarray(xT[b][:, cols])
        in_maps.append({
            "xT": xT[b],
            "xn": xr[b],
            "xqT": xqT,
            "wm": wm, "wvT": wvT,
            "masks": mask_A if role == 0 else mask_B,
            "ones": ones,
        })
    return in_maps


def assemble(results):
    out = np.empty((B, T, C), np.float32)
    for core in range(8):
        b, role = divmod(core, 2)
        o = results[core]["oo"]                      # [1024, C], strip-major
        rparts = results[core]["rows"]               # [QS, 1024] pair partials
        rsum = np.zeros(QS * SW, np.float64)
        for pair in range(QS):
            q0 = 256 * pair                          # valid cols for this pair
            rsum[q0:] += rparts[pair, q0:]
        o = o / rsum[:, None].astype(np.float32)
        for j in range(QS):
            r0 = 512 * j + SW * role
            out[b, r0:r0 + SW] = o[j * SW:(j + 1) * SW]
    return out


def kernel(x, Wq, Wk, Wv):
    nc = _get_nc()
    in_maps = make_in_maps(x, Wq, Wk, Wv)
    res = bass_utils.run_bass_kernel_spmd(nc, in_maps, core_ids=list(range(8)))
    return assemble(res.results)


def _install_trace_shim():
    """Provide antenv.axon_hooks (absent in this image) so trace=True works."""
    import sys
    import types
    if "antenv.axon_hooks" in sys.modules:
        return
    hook_box = [None]
    mod = types.ModuleType("antenv.axon_hooks")
    mod.set_axon_ntff_profile_hook = lambda h: hook_box.__setitem__(0, h)
    mod.get_axon_ntff_profile_hook = lambda: hook_box[0]
    import antenv
    sys.modules["antenv.axon_hooks"] = mod
    antenv.axon_hooks = mod
    try:
        from trn_agent_boot.trn_boot import _ntff_profile_via_ctypes
        mod.set_axon_ntff_profile_hook(
            _ntff_profile_via_ctypes("/opt/axon/libaxon_pjrt.so"))
    except Exception:
        pass


def run_traced(x, Wq, Wk, Wv):
    """Like kernel() but with NTFF tracing; returns (out, BassKernelResults)."""
    _install_trace_shim()
    nc = _get_nc()
    in_maps = make_in_maps(x, Wq, Wk, Wv)
    res = bass_utils.run_bass_kernel_spmd(
        nc, in_maps, core_ids=list(range(8)), trace=True,
        trace_cores=list(range(8)))
    return assemble(res.results), res


# revision 8
# speedup vs baseline: 1.0478x; 1.0478x over previous

"""Causal attention (no head split) on 8 trn2 NeuronCores — v3.

Reference computation (per batch b):
    q = x @ Wq^T ; k = x @ Wk^T ; v = x @ Wv^T          (nn.Linear convention)
    wei = softmax(mask(q @ k^T / sqrt(C)))               (causal)
    out = wei @ v

Algebraic restructuring (K and V are never materialized):
    S   = q k^T = x (Wq^T Wk) x^T = x M x^T     with M precomputed on host
    out = wei v = (wei x) Wv^T
Device computes, all PE operands bf16:
    G^T = M^T xq^T                   (projection of this core's queries)
    S^T[s,t] = xT(lhsT) G^T(rhs)     (contract over C)
    P^T = exp(S^T / 32) * mask ; per-chunk rowsum partials -> DRAM
    H[c,t] += xn(lhsT) P^T(rhs)      (contract over s, accumulated in SBUF)
    O[q,d]  = hr(lhsT) Wv^T(rhs)     (contract over c, natural-layout out)
Final softmax normalization (divide by summed rowsum) happens on the host.

Sharding: 2 cores per batch (B=4). Queries split into eight 256-row strips;
role A takes strips {0,2,4,6} (rows [512j,512j+256)), role B {1,3,5,7}.
Every core runs the IDENTICAL instruction stream (single SPMD NEFF); role
differences are carried entirely by input data (query columns + mask tiles).

v3 layout: key chunks processed in natural order; for chunk c the
participating query strips are a CONTIGUOUS column range [256*(c//2), 1024),
processed in strip-aligned pieces of width <= 512 so S/H matmuls run at
N=512 where possible.  H-accumulation adds alternate Vector/GpSimd.  The
G phase is split (strips 0,1 / strips 2,3) and interleaved with the first
chunks to hide DMA pacing.  Per-strip Wv projections run at the strip's
last chunk (odd c), spreading output copies/DMA through the kernel.
"""
import os
import numpy as np

import concourse.bass as bass
from concourse import bacc
import concourse.mybir as mybir
from concourse.tile import TileContext
from concourse import bass_utils

B, T, C = 4, 2048, 1024
P = 128
CS = C // P          # 8 contraction subtiles
NCH = T // 256       # 8 kv chunks of 256
QS = 4               # query strips per core
SW = 256             # strip width
SCALE = 1.0 / np.sqrt(C)  # 1/32

BF16 = mybir.dt.bfloat16
F32 = mybir.dt.float32


def chunk_pieces(c):
    """Strip-aligned pieces (q0, width<=512) covering [256*(c//2), 1024)."""
    q0 = 256 * (c // 2)
    if q0 == 0:
        return [(0, 512), (512, 512)]
    if q0 == 256:
        return [(256, 256), (512, 512)]
    if q0 == 512:
        return [(512, 512)]
    return [(768, 256)]


def build():
    nc = bacc.Bacc(trn_type="TRN2", name="causal_attn_v3")
    xT = nc.dram_tensor("xT", [C, T], BF16, kind="ExternalInput")    # x^T (batch)
    xn = nc.dram_tensor("xn", [T, C], BF16, kind="ExternalInput")    # x natural
    xqT = nc.dram_tensor("xqT", [C, QS * SW], BF16, kind="ExternalInput")
    # M = Wq^T Wk, host-tiled [ds, h, p, c4, d] so arrival is ds-progressive
    wmt = nc.dram_tensor("wmt", [CS * 2 * P * 4, P], BF16, kind="ExternalInput")
    wvT = nc.dram_tensor("wvT", [C, C], BF16, kind="ExternalInput")  # Wv^T [c,d]
    masks = nc.dram_tensor("masks", [P, 4, SW], BF16, kind="ExternalInput")
    ones = nc.dram_tensor("ones", [P, 1], BF16, kind="ExternalInput")
    oo = nc.dram_tensor("oo", [QS * 2 * P, C], BF16, kind="ExternalOutput")
    rows = nc.dram_tensor("rows", [QS, QS * SW], F32, kind="ExternalOutput")

    xT_r = xT.rearrange("(cs p) t -> p cs t", p=P)
    xn_r = xn.rearrange("(ch ss p) c -> p ch ss c", p=P, ss=2)
    xqT_r = xqT.rearrange("(cs p) t -> p cs t", p=P)
    wmt_r = wmt.rearrange("(ds h p c4) d -> p ds h c4 d", h=2, p=P, c4=4)
    wvT_r = wvT.rearrange("(cs p) d -> p cs d", p=P)
    oo_r = oo.rearrange("(j qh p) d -> p j qh d", qh=2, p=P)

    with TileContext(nc) as tc:
        with tc.tile_pool(name="psS", bufs=4, space="PSUM") as psS, \
             tc.tile_pool(name="psH", bufs=3, space="PSUM") as psH, \
             tc.tile_pool(name="psR", bufs=1, space="PSUM") as psR, \
             tc.tile_pool(name="keep", bufs=1) as keep, \
             tc.tile_pool(name="wpool", bufs=2) as wpool, \
             tc.tile_pool(name="stream", bufs=8) as stream, \
             tc.tile_pool(name="hrpool", bufs=2) as hrpool, \
             tc.tile_pool(name="ppool", bufs=4) as ppool, \
             tc.tile_pool(name="opool", bufs=3) as opool:

            gT = keep.tile([P, CS, QS * SW], BF16, tag="gT")
            hh = keep.tile([P, CS, QS * SW], F32, tag="hh")
            msk = keep.tile([P, 4, SW], BF16, tag="msk")
            ones_t = keep.tile([P, 1], BF16, tag="ones")

            # ---- DMA triggers cost ~650ns each (serialized per engine), so:
            # few large calls, split across the two HWDGE engines (sync,
            # scalar).  wm lands ds-pair-wise (pre-tiled layout); G consumes
            # in ds order.
            wq2 = wpool.tile([P, CS, 2, 4, P], BF16, tag="w")
            for k in range(4):
                nc.sync.dma_start(wq2[:, 2 * k:2 * k + 2], wmt_r[:, 2 * k:2 * k + 2])
            xq = keep.tile([P, CS, QS * SW], BF16, tag="xq")
            nc.sync.dma_start(xq[:, :, 0:256], xqT_r[:, :, 0:256])
            nc.sync.dma_start(msk[:], masks[:])
            nc.sync.dma_start(ones_t[:], ones[:])
            nc.sync.dma_start(xq[:, :, 256:512], xqT_r[:, :, 256:512])
            nc.sync.dma_start(xq[:, :, 512:1024], xqT_r[:, :, 512:1024])

            wv = wpool.tile([P, CS, C], BF16, tag="w")
            for dh in range(2):
                nc.sync.dma_start(wv[:, :, dh * 512:(dh + 1) * 512],
                                    wvT_r[:, :, dh * 512:(dh + 1) * 512])

            def g_phase(j):
                # gT[:, ds, 256j:+256] = M^T xq^T for strip j
                for ds in range(CS):
                    pq = psH.tile([P, SW], F32, tag="po")
                    for cs in range(CS):
                        nc.tensor.matmul(
                            pq[:], wq2[:, ds, cs // 4, cs % 4],
                            xq[:, cs, j * SW:(j + 1) * SW],
                            start=(cs == 0), stop=(cs == CS - 1))
                    nc.scalar.copy(gT[:, ds, j * SW:(j + 1) * SW], pq[:])

            def load_chunk(c):
                xt = stream.tile([P, CS, 256], BF16, tag="xt")
                nc.sync.dma_start(xt[:], xT_r[:, :, c * 256:(c + 1) * 256])
                xna = stream.tile([P, 2, C], BF16, tag="xn")
                nc.sync.dma_start(xna[:], xn_r[:, c])
                return xt, xna

            def s_piece(c, xt, q0, w, first_piece):
                # S^T then P^T = exp(S/32) [* mask on the diagonal 256 cols]
                pT = ppool.tile([P, 2, w], BF16, tag="pT")
                for ss in range(2):
                    st = psS.tile([P, w], F32, tag="st")
                    for cs in range(CS):
                        nc.tensor.matmul(
                            st[:], xt[:, cs, ss * P:(ss + 1) * P],
                            gT[:, cs, q0:q0 + w],
                            start=(cs == 0), stop=(cs == CS - 1))
                    nc.scalar.activation(
                        pT[:, ss], st[:],
                        mybir.ActivationFunctionType.Exp, scale=float(SCALE))
                if first_piece:
                    # first 256 cols of the first piece = diagonal strip c//2
                    midx = c % 2
                    nc.gpsimd.tensor_mul(
                        pT[:, :, 0:SW], pT[:, :, 0:SW],
                        msk[:, midx * 2:midx * 2 + 2])
                return pT

            def h_pair(pair, xna0, xna1, pT0, pT1, q0, w):
                # rowsum partials for the chunk pair -> DRAM (summed on host)
                rw = psR.tile([1, w], F32, tag="rw")
                for i, pT in enumerate((pT0, pT1)):
                    for ss in range(2):
                        nc.tensor.matmul(
                            rw[:], ones_t[:], pT[:, ss],
                            start=(i == 0 and ss == 0),
                            stop=(i == 1 and ss == 1))
                rsb = opool.tile([1, w], F32, tag="rsb")
                nc.scalar.copy(rsb[:], rw[:])
                nc.sync.dma_start(rows[pair:pair + 1, q0:q0 + w], rsb[:])

                # H += x P^T accumulated over both chunks of the pair in PSUM
                for cs in range(CS):
                    po = psH.tile([P, w], F32, tag="po")
                    for i, (xna, pT) in enumerate(((xna0, pT0), (xna1, pT1))):
                        for ss in range(2):
                            nc.tensor.matmul(
                                po[:], xna[:, ss, cs * P:(cs + 1) * P],
                                pT[:, ss],
                                start=(i == 0 and ss == 0),
                                stop=(i == 1 and ss == 1))
                    hsl = hh[:, cs, q0:q0 + w]
                    if pair == 0:
                        nc.vector.tensor_copy(hsl, po[:])
                    else:
                        nc.vector.tensor_add(hsl, hsl, po[:])

            def project(j):
                # O[q, d] = hr(lhsT) @ Wv^T(rhs); hr cast per cs-half so the
                # first contraction steps start before the second half lands
                hr = hrpool.tile([P, CS, SW], BF16, tag="hr")
                tsl = slice(j * SW, (j + 1) * SW)
                for h2 in range(2):
                    nc.scalar.copy(hr[:, 4 * h2:4 * h2 + 4],
                                   hh[:, 4 * h2:4 * h2 + 4, tsl])
                for qh in range(2):
                    for dp in range(2):
                        ps = psH.tile([P, 512], F32, tag="po")
                        for cs in range(CS):
                            nc.tensor.matmul(
                                ps[:], hr[:, cs, qh * P:(qh + 1) * P],
                                wv[:, cs, dp * 512:(dp + 1) * 512],
                                start=(cs == 0), stop=(cs == CS - 1))
                        ost = opool.tile([P, 512], BF16, tag="ost")
                        nc.scalar.copy(ost[:], ps[:])
                        nc.sync.dma_start(
                            oo_r[:, j, qh, dp * 512:(dp + 1) * 512], ost[:])

            # ---- schedule: chunk pairs (strip boundaries align to pairs);
            # G(0,1) | pair0-A | G(2,3) | projJ0 | pair0-B | pair1 | projJ1 |
            # pair2 | projJ2 | pair3 | projJ3
            g_phase(0)
            g_phase(1)
            xt0, xna0 = load_chunk(0)
            xt1, xna1 = load_chunk(1)
            pA0 = s_piece(0, xt0, 0, 512, True)
            pA1 = s_piece(1, xt1, 0, 512, True)
            h_pair(0, xna0, xna1, pA0, pA1, 0, 512)
            g_phase(2)
            g_phase(3)
            project(0)
            pB0 = s_piece(0, xt0, 512, 512, False)
            pB1 = s_piece(1, xt1, 512, 512, False)
            h_pair(0, xna0, xna1, pB0, pB1, 512, 512)
            for pair in (1, 2, 3):
                c0, c1 = 2 * pair, 2 * pair + 1
                xta, xnaa = load_chunk(c0)
                xtb, xnab = load_chunk(c1)
                for (q0, w) in chunk_pieces(c0):
                    first = (q0 == 256 * pair)
                    pa = s_piece(c0, xta, q0, w, first)
                    pb = s_piece(c1, xtb, q0, w, first)
                    h_pair(pair, xnaa, xnab, pa, pb, q0, w)
                project(pair)

    nc.compile()
    return nc


_NC = None


def _get_nc():
    global _NC
    if _NC is None:
        _NC = build()
    return _NC


def make_in_maps(x, Wq, Wk, Wv):
    import ml_dtypes
    bf16 = ml_dtypes.bfloat16
    x = np.asarray(x, dtype=np.float32)
    wq64 = np.asarray(Wq, np.float64)
    wk64 = np.asarray(Wk, np.float64)
    wm = (wq64.T @ wk64).astype(bf16)                        # M = Wq^T Wk [c',c]
    # tile to [ds, h, p, c4, d]: c = (h*4+c4)*128+p (partition), d = ds*128+dd
    wmt = np.ascontiguousarray(
        wm.reshape(2, 4, P, CS, P).transpose(3, 0, 2, 1, 4)).reshape(CS * 2 * P * 4, P)
    wvT = np.ascontiguousarray(np.asarray(Wv, np.float32).T).astype(bf16)
    ones = np.ones((P, 1), bf16)

    # mask tiles [p, midx*2+ss, t]: tri = 1 if (ss*128+p) <= t
    s_idx = (np.arange(2)[:, None, None] * P + np.arange(P)[None, :, None])
    tri = (s_idx <= np.arange(SW)[None, None, :]).astype(np.float32)
    tri = np.ascontiguousarray(tri.transpose(1, 0, 2))
    zeros = np.zeros((P, 2, SW), np.float32)
    ones2 = np.ones((P, 2, SW), np.float32)
    mask_A = np.ascontiguousarray(np.concatenate([tri, zeros], axis=1)).astype(bf16)
    mask_B = np.ascontiguousarray(np.concatenate([ones2, tri], axis=1)).astype(bf16)

    xr = [x[b].astype(bf16) for b in range(B)]
    xT = [np.ascontiguousarray(xr[b].T) for b in range(B)]
    in_maps = []
    for core in range(8):
        b, role = divmod(core, 2)
        cols = np.concatenate(
            [np.arange(512 * j + SW * role, 512 * j + SW * role + SW)
             for j in range(QS)])
        xqT = np.ascontiguousarray(xT[b][:, cols])
        in_maps.append({
            "xT": xT[b],
            "xn": xr[b],
            "xqT": xqT,
            "wmt": wmt, "wvT": wvT,
            "masks": mask_A if role == 0 else mask_B,
            "ones": ones,
        })
    return in_maps


def assemble(results):
    out = np.empty((B, T, C), np.float32)
    for core in range(8):
        b, role = divmod(core, 2)
        o = results[core]["oo"].astype(np.float32)   # [1024, C], strip-major
        rparts = results[core]["rows"]               # [QS, 1024] pair partials
        rsum = np.zeros(QS * SW, np.float64)
        for pair in range(QS):
            q0 = 256 * pair                          # valid cols for this pair
            rsum[q0:] += rparts[pair, q0:]
        o = o / rsum[:, None].astype(np.float32)
        for j in range(QS):
            r0 = 512 * j + SW * role
            out[b, r0:r0 + SW] = o[j * SW:(j + 1) * SW]
    return out


def kernel(x, Wq, Wk, Wv):
    nc = _get_nc()
    in_maps = make_in_maps(x, Wq, Wk, Wv)
    res = bass_utils.run_bass_kernel_spmd(nc, in_maps, core_ids=list(range(8)))
    return assemble(res.results)


def _install_trace_shim():
    """Provide antenv.axon_hooks (absent in this image) so trace=True works."""
    import sys
    import types
    if "antenv.axon_hooks" in sys.modules:
        return
    hook_box = [None]
    mod = types.ModuleType("antenv.axon_hooks")
    mod.set_axon_ntff_profile_hook = lambda h: hook_box.__setitem__(0, h)
    mod.get_axon_ntff_profile_hook = lambda: hook_box[0]
    import antenv
    sys.modules["antenv.axon_hooks"] = mod
    antenv.axon_hooks = mod
    try:
        from trn_agent_boot.trn_boot import _ntff_profile_via_ctypes
        mod.set_axon_ntff_profile_hook(
            _ntff_profile_via_ctypes("/opt/axon/libaxon_pjrt.so"))
    except Exception:
        pass


def run_traced(x, Wq, Wk, Wv):
    """Like kernel() but with NTFF tracing; returns (out, BassKernelResults)."""
    _install_trace_shim()
    nc = _get_nc()
    in_maps = make_in_maps(x, Wq, Wk, Wv)
    res = bass_utils.run_bass_kernel_spmd(
        nc, in_maps, core_ids=list(range(8)), trace=True,
        trace_cores=list(range(8)))
    return assemble(res.results), res
